# revision 10
# baseline (speedup 1.0000x reference)
"""Trainium2 Bass kernel for nn_CRLoss (masked cosine-similarity contrastive loss).

Strategy (data-parallel over batch, 2 batches per core on 8 cores):
  Host: permute each batch's rows so label==0 ("fake") rows come first, then
  label==1 ("real") rows; ship transposed embeddings E^T [D, T] per batch.
  Device (per batch): normalize rows (norm^2 via ones-matmul column sums, then
  exp(-0.5*ln(x)) on ACT), compute S = N^T.T @ N^T row-tile by row-tile on the
  PE (16 x [128, 2048] fp32 tiles in PSUM), and reduce each row-tile on DVE:
  min/max over the compile-time "certain" column zones [0:CF) / [CR:T), plus a
  narrow data-dependent mixed zone [CF:CR) handled with +16.0 bias masks so a
  single biased tile yields both the min of the included class and (value-16)
  the max of the excluded class.
  Host: combine certain/mixed stats, apply the reference's relu/mean/sum tail
  on the [T]-sized per-row statistics.
"""
import os
import sys

sys.path.insert(0, "/opt/trn_rl_repo")

import numpy as np

B, T, D = 16, 2048, 128
NCORES = 8
BPC = B // NCORES  # batches per core
BIG = 16.0  # bias magnitude; sims are in [-1, 1] so +-16 always dominates
TH_SIM_MIN = 0.9
TH_DIFF_MAX = 0.1


def _build(CF, CR, t_lo, t_hi):
    import concourse.bacc as bacc
    import concourse.mybir as mybir
    import concourse.tile as tile

    f32 = mybir.dt.float32
    bf16 = mybir.dt.bfloat16
    Alu = mybir.AluOpType
    Act = mybir.ActivationFunctionType
    X = mybir.AxisListType.X
    MW = CR - CF
    NT128 = T // 128  # row tiles per batch
    NT512 = T // 512  # psum-bank chunks per row tile

    # Force every ACT load to the one table set containing ln+exp+square+copy;
    # walrus/bacc otherwise thrash between per-function sets (~1.3us per load).
    if not getattr(bacc, "_crl_act_patch", False):
        _orig_tables = bacc.get_activation_tables

        def _one_set(arch):
            return {
                name: (fns if name == "natural_log_exp_and_others" else set())
                for name, fns in _orig_tables(arch).items()
            }

        bacc.get_activation_tables = _one_set
        bacc._crl_act_patch = True

    nc = bacc.Bacc("TRN2", target_bir_lowering=False)
    embT = nc.dram_tensor("embt", [BPC, 128, T], f32, kind="ExternalInput")
    masks = nc.dram_tensor("masks", [BPC, 128, 2, MW], f32, kind="ExternalInput")
    stats_c = nc.dram_tensor("stats_c", [BPC, 128, 4, NT128], f32, kind="ExternalOutput")
    stats_m = nc.dram_tensor("stats_m", [BPC, 128, 4, NT128], f32, kind="ExternalOutput")

    import concourse.bass as bass

    def bcast_mid(ap2d, n):
        # [P, M] AP -> [P, n, M] with stride-0 middle dim (free-dim broadcast)
        return bass.AP(
            ap2d.tensor, ap2d.offset, [list(ap2d.ap[0]), [0, n], list(ap2d.ap[1])]
        )

    with tile.TileContext(nc) as tc:
        with (
            tc.tile_pool(name="cst", bufs=1) as cst,
            tc.tile_pool(name="sb", bufs=2) as sb,
            tc.tile_pool(name="scr", bufs=3) as scrp,
            tc.tile_pool(name="st", bufs=2) as stp,
            tc.tile_pool(name="ps", bufs=2, space="PSUM") as ps,
        ):
            ones = cst.tile([128, 128], bf16)
            nc.gpsimd.memset(ones[:], 1.0)

            # Phase A (both slots): normalized bf16 N^T tiles, pipelined in
            # 512-col chunks (separate tiles per chunk for fine-grained deps):
            # DMA -> gpsimd square -> ones-matmul colsum-bcast -> ACT ln ->
            # ACT exp(-0.5*x) -> gpsimd mult. Ln/Exp share one ACT table set.
            nts, mks = [], []
            for s in range(BPC):
                mk = cst.tile([128, 2, MW], f32, tag=f"mk{s}")
                nc.sync.dma_start(mk[:], masks[s])
                mks.append(mk)
                pbc = ps.tile([128, T], f32, tag="ps")
                sbts = []
                etc = []
                for c in range(NT512):
                    col = slice(c * 512, (c + 1) * 512)
                    et = sb.tile([128, 512], f32, tag=f"et{s}c{c}")
                    nc.sync.dma_start(et[:], embT[s][:, col])
                    etc.append(et)
                    sq = sb.tile([128, 512], bf16, tag=f"sq{c}")
                    nc.gpsimd.tensor_tensor(sq[:], et[:], et[:], op=Alu.mult)
                    nc.tensor.matmul(pbc[:, col], ones[:], sq[:])
                    # 1/sqrt(x) = exp(-0.5*ln(x)) (Rsqrt is banned for accuracy)
                    lg = sb.tile([128, 512], f32, tag=f"lg{c}")
                    nc.scalar.activation(lg[:], pbc[:, col], Act.Ln)
                    sbt = sb.tile([128, 512], f32, tag=f"sbt{s}c{c}")
                    nc.scalar.activation(sbt[:], lg[:], Act.Exp, scale=-0.5)
                    sbts.append(sbt)
                ntc = []
                for c in range(NT512):
                    nt = cst.tile([128, 512], bf16, tag=f"nt{s}c{c}")
                    nc.vector.tensor_tensor(nt[:], etc[c][:], sbts[c][:], op=Alu.mult)
                    ntc.append(nt)
                nts.append(ntc)

            # Phase B: per slot, Gram row-tiles + reductions.
            # Mixed zone [CF:CR) is copied (ACT) per row-tile into two stacks,
            # then bias-added + reduced in a few big DVE ops at slot end:
            #   stackA + 16*real -> v0 = min, v1 = max (host subtracts 16)
            #   stackB + 16*fake -> v2 = min, v3 = max (host subtracts 16)
            NA = t_hi          # tiles 0..t_hi-1 need v0/v1 (fake or straddle rows)
            NB = NT128 - t_lo  # tiles t_lo..15 need v2/v3 (real or straddle rows)
            for s in range(BPC):
                ntc, mk = nts[s], mks[s]
                stc = stp.tile([128, 4, NT128], f32, tag="stc")
                stm = stp.tile([128, 4, NT128], f32, tag="stm")
                stkA = stp.tile([128, NA, MW], f32, tag="stkA")
                stkB = stp.tile([128, NB, MW], f32, tag="stkB")

                for rt in range(NT128):
                    pS = ps.tile([128, T], f32, tag="ps")
                    lhsT = ntc[rt // 4][:, (rt % 4) * 128 : (rt % 4 + 1) * 128]
                    for j in range(NT512):
                        nc.tensor.matmul(
                            pS[:, j * 512 : (j + 1) * 512],
                            lhsT,
                            ntc[j][:],
                        )
                    fake_rows = rt < t_lo
                    real_rows = rt >= t_hi
                    # certain-zone reductions
                    if not real_rows:  # fake or straddle rows: v0, v1
                        nc.vector.tensor_reduce(
                            stc[:, 0, rt : rt + 1], pS[:, 0:CF], axis=X, op=Alu.min
                        )
                        nc.vector.tensor_reduce(
                            stc[:, 1, rt : rt + 1], pS[:, CR:T], axis=X, op=Alu.max
                        )
                        nc.scalar.copy(stkA[:, rt, :], pS[:, CF:CR])
                    if not fake_rows:  # real or straddle rows: v2, v3
                        nc.vector.tensor_reduce(
                            stc[:, 2, rt : rt + 1], pS[:, CR:T], axis=X, op=Alu.min
                        )
                        nc.vector.tensor_reduce(
                            stc[:, 3, rt : rt + 1], pS[:, 0:CF], axis=X, op=Alu.max
                        )
                        nc.scalar.copy(stkB[:, rt - t_lo, :], pS[:, CF:CR])

                nc.vector.tensor_tensor(
                    stkA[:], stkA[:], bcast_mid(mk[:, 0, :], NA), op=Alu.add
                )
                nc.vector.tensor_reduce(
                    stm[:, 0, 0:NA], stkA[:], axis=X, op=Alu.min
                )
                nc.vector.tensor_reduce(
                    stm[:, 1, 0:NA], stkA[:], axis=X, op=Alu.max
                )
                nc.vector.tensor_tensor(
                    stkB[:], stkB[:], bcast_mid(mk[:, 1, :], NB), op=Alu.add
                )
                nc.vector.tensor_reduce(
                    stm[:, 2, t_lo:NT128], stkB[:], axis=X, op=Alu.min
                )
                nc.vector.tensor_reduce(
                    stm[:, 3, t_lo:NT128], stkB[:], axis=X, op=Alu.max
                )

                nc.gpsimd.dma_start(stats_c[s], stc[:])
                nc.gpsimd.dma_start(stats_m[s], stm[:])

    nc.compile()
    return nc


def kernel(embeddings, label):
    embeddings = np.ascontiguousarray(np.asarray(embeddings, dtype=np.float32))
    label = np.asarray(label)
    assert embeddings.shape == (B, T, D) and label.shape == (B, T)

    # host-side packing: fake (label 0) rows first, per batch
    perms = np.empty((B, T), dtype=np.int64)
    nfs = np.empty(B, dtype=np.int64)
    for b in range(B):
        lb = label[b]
        perms[b] = np.argsort(lb, kind="stable")
        nfs[b] = int((lb == 0).sum())
    valid = (nfs > 0) & (nfs < T)
    if not valid.any():
        return np.float32(0.0)

    CF = int(nfs[valid].min())
    CR = int(nfs[valid].max())
    # invalid batches still run through the device with garbage-safe ranges;
    # clamp so all slices are non-empty
    CF = max(8, min(CF, T - 8))
    CR = max(CF + 1, min(CR, T - 8))
    MW = CR - CF
    t_lo = CF // 128
    t_hi = (CR + 127) // 128

    nc = _build(CF, CR, t_lo, t_hi)

    # per-core inputs
    in_maps = []
    packedE = np.empty((B, 128, T), dtype=np.float32)
    for b in range(B):
        packedE[b] = embeddings[b][perms[b]].T  # [D, T]
    for c in range(NCORES):
        embt = np.empty((BPC, 128, T), dtype=np.float32)
        mks = np.zeros((BPC, 128, 2, MW), dtype=np.float32)
        for s in range(BPC):
            b = c * BPC + s
            embt[s] = packedE[b]
            lb_packed = label[b][perms[b]]  # 0..0 1..1
            mz = lb_packed[CF:CR]
            mks[s, :, 0, :] = np.where(mz == 1, BIG, 0.0)[None, :]  # biased_A: +16 on real
            mks[s, :, 1, :] = np.where(mz == 0, BIG, 0.0)[None, :]  # biased_B: +16 on fake
        in_maps.append({"embt": embt, "masks": mks})

    from concourse.bass_utils import run_bass_kernel_spmd

    trace = bool(os.environ.get("CRL_TRACE"))
    if trace:
        _install_ntff_shim()
    res = run_bass_kernel_spmd(
        nc, in_maps, core_ids=list(range(NCORES)), trace=trace
    )
    if trace and res.exec_time_ns is not None:
        print(f"HW exec time: {res.exec_time_ns} ns")
        if res.instructions_and_trace:
            print("trace:", res.instructions_and_trace[1])

    # host tail on [T]-sized stats
    total = 0.0
    for c in range(NCORES):
        out = res.results[c]
        for s in range(BPC):
            b = c * BPC + s
            if not valid[b]:
                continue
            nf = int(nfs[b])
            stc = out["stats_c"][s].astype(np.float64)  # [128, 4, NT]
            stm = out["stats_m"][s].astype(np.float64)
            # row r = t*128 + p  ->  [4, T]
            sc = stc.transpose(1, 2, 0).reshape(4, T)
            sm = stm.transpose(1, 2, 0).reshape(4, T)
            minfake = np.minimum(sc[0], sm[0])
            maxreal = np.maximum(sc[1], sm[1] - BIG)
            minreal = np.minimum(sc[2], sm[2])
            maxfake = np.maximum(sc[3], sm[3] - BIG)
            f2f = np.maximum(TH_SIM_MIN - minfake[:nf], 0.0).mean()
            r2r = np.maximum(TH_SIM_MIN - minreal[nf:], 0.0).mean()
            f2r = np.maximum(maxreal[:nf] - TH_DIFF_MAX, 0.0).mean()
            r2f = np.maximum(maxfake[nf:] - TH_DIFF_MAX, 0.0).mean()
            total += f2f + r2r + f2r + r2f
    return np.float32(total / B)


def _install_ntff_shim():
    """antenv.axon_hooks is missing on this image; inject it so trace=True works."""
    import types

    import antenv

    if hasattr(antenv, "axon_hooks"):
        return
    from trn_agent_boot.trn_boot import _ntff_profile_via_ctypes

    mod = types.ModuleType("antenv.axon_hooks")
    mod._hook = _ntff_profile_via_ctypes("/opt/axon/libaxon_pjrt.so")
    mod.get_axon_ntff_profile_hook = lambda: mod._hook
    mod.set_axon_ntff_profile_hook = lambda h: setattr(mod, "_hook", h)
    sys.modules["antenv.axon_hooks"] = mod
    antenv.axon_hooks = mod


# revision 15
# speedup vs baseline: 1.0050x; 1.0050x over previous
"""Trainium2 Bass kernel for nn_CRLoss (masked cosine-similarity contrastive loss).

Strategy (data-parallel over batch, 2 batches per core on 8 cores):
  Host: permute each batch's rows so label==0 ("fake") rows come first, then
  label==1 ("real") rows; ship transposed embeddings E^T [D, T] per batch.
  Device (per batch): normalize rows (norm^2 via ones-matmul column sums, then
  exp(-0.5*ln(x)) on ACT), compute S = N^T.T @ N^T row-tile by row-tile on the
  PE (16 x [128, 2048] fp32 tiles in PSUM), and reduce each row-tile on DVE:
  min/max over the compile-time "certain" column zones [0:CF) / [CR:T), plus a
  narrow data-dependent mixed zone [CF:CR) handled with +16.0 bias masks so a
  single biased tile yields both the min of the included class and (value-16)
  the max of the excluded class.
  Host: combine certain/mixed stats, apply the reference's relu/mean/sum tail
  on the [T]-sized per-row statistics.
"""
import os
import sys

sys.path.insert(0, "/opt/trn_rl_repo")

import numpy as np

B, T, D = 16, 2048, 128
NCORES = 8
BPC = B // NCORES  # batches per core
BIG = 16.0  # bias magnitude; sims are in [-1, 1] so +-16 always dominates
TH_SIM_MIN = 0.9
TH_DIFF_MAX = 0.1


def _build(CF, CR, t_lo, t_hi):
    import concourse.bacc as bacc
    import concourse.mybir as mybir
    import concourse.tile as tile

    f32 = mybir.dt.float32
    bf16 = mybir.dt.bfloat16
    Alu = mybir.AluOpType
    Act = mybir.ActivationFunctionType
    X = mybir.AxisListType.X
    MW = CR - CF
    NT128 = T // 128  # row tiles per batch
    NT512 = T // 512  # psum-bank chunks per row tile

    # Force every ACT load to the one table set containing ln+exp+square+copy;
    # walrus/bacc otherwise thrash between per-function sets (~1.3us per load).
    if not getattr(bacc, "_crl_act_patch", False):
        _orig_tables = bacc.get_activation_tables

        def _one_set(arch):
            return {
                name: (fns if name == "natural_log_exp_and_others" else set())
                for name, fns in _orig_tables(arch).items()
            }

        bacc.get_activation_tables = _one_set
        bacc._crl_act_patch = True

    nc = bacc.Bacc("TRN2", target_bir_lowering=False)
    embT = nc.dram_tensor("embt", [BPC, 128, T], f32, kind="ExternalInput")
    masks = nc.dram_tensor("masks", [BPC, 128, 2, MW], f32, kind="ExternalInput")
    stats_c = nc.dram_tensor("stats_c", [BPC, 128, 4, NT128], f32, kind="ExternalOutput")
    stats_m = nc.dram_tensor("stats_m", [BPC, 128, 4, NT128], f32, kind="ExternalOutput")

    import concourse.bass as bass

    def bcast_mid(ap2d, n):
        # [P, M] AP -> [P, n, M] with stride-0 middle dim (free-dim broadcast)
        return bass.AP(
            ap2d.tensor, ap2d.offset, [list(ap2d.ap[0]), [0, n], list(ap2d.ap[1])]
        )

    with tile.TileContext(nc) as tc:
        with (
            tc.tile_pool(name="cst", bufs=1) as cst,
            tc.tile_pool(name="sb", bufs=2) as sb,
            tc.tile_pool(name="scr", bufs=3) as scrp,
            tc.tile_pool(name="st", bufs=2) as stp,
            tc.tile_pool(name="ps", bufs=2, space="PSUM") as ps,
        ):
            ones = cst.tile([128, 128], bf16)
            nc.gpsimd.memset(ones[:], 1.0)

            # Phase A (both slots): normalized bf16 N^T tiles, pipelined in
            # 512-col chunks (separate tiles per chunk for fine-grained deps):
            # DMA -> gpsimd square -> ones-matmul colsum-bcast -> ACT ln ->
            # ACT exp(-0.5*x) -> gpsimd mult. Ln/Exp share one ACT table set.
            nts, mks = [], []
            for s in range(BPC):
                mk = cst.tile([128, 2, MW], f32, tag=f"mk{s}")
                nc.sync.dma_start(mk[:], masks[s])
                mks.append(mk)
                pbc = [
                    ps.tile([128, 1024], f32, tag="ph", name=f"pbc{s}_{h}")
                    for h in range(2)
                ]
                sbts = []
                etc = []
                for c in range(NT512):
                    col = slice(c * 512, (c + 1) * 512)
                    hcol = slice((c % 2) * 512, (c % 2) * 512 + 512)
                    et = sb.tile([128, 512], f32, tag=f"et{s}c{c}")
                    nc.sync.dma_start(et[:], embT[s][:, col])
                    etc.append(et)
                    sq = sb.tile([128, 512], bf16, tag=f"sq{c}")
                    nc.gpsimd.tensor_tensor(sq[:], et[:], et[:], op=Alu.mult)
                    nc.tensor.matmul(pbc[c // 2][:, hcol], ones[:], sq[:])
                    # 1/sqrt(x) = exp(-0.5*ln(x)) (Rsqrt is banned for accuracy)
                    lg = sb.tile([128, 512], f32, tag=f"lg{c}")
                    nc.scalar.activation(lg[:], pbc[c // 2][:, hcol], Act.Ln)
                    sbt = sb.tile([128, 512], f32, tag=f"sbt{s}c{c}")
                    nc.scalar.activation(sbt[:], lg[:], Act.Exp, scale=-0.5)
                    sbts.append(sbt)
                ntc = []
                for c in range(NT512):
                    nt = cst.tile([128, 512], bf16, tag=f"nt{s}c{c}")
                    eng = nc.vector if s == 0 else nc.gpsimd
                    eng.tensor_tensor(nt[:], etc[c][:], sbts[c][:], op=Alu.mult)
                    ntc.append(nt)
                nts.append(ntc)

            # Phase B: per slot, Gram row-tiles + reductions.
            # Mixed zone [CF:CR) is copied (ACT) per row-tile into two stacks,
            # then bias-added + reduced in a few big DVE ops at slot end:
            #   stackA + 16*real -> v0 = min, v1 = max (host subtracts 16)
            #   stackB + 16*fake -> v2 = min, v3 = max (host subtracts 16)
            NA = t_hi          # tiles 0..t_hi-1 need v0/v1 (fake or straddle rows)
            NB = NT128 - t_lo  # tiles t_lo..15 need v2/v3 (real or straddle rows)
            for s in range(BPC):
                ntc, mk = nts[s], mks[s]
                stc = stp.tile([128, 4, NT128], f32, tag="stc")
                stm = stp.tile([128, 4, NT128], f32, tag="stm")
                stkA = stp.tile([128, NA, MW], f32, tag="stkA")
                stkB = stp.tile([128, NB, MW], f32, tag="stkB")

                WL = 1024 - CF  # mixed-zone cols in the low psum half
                WH = CR - 1024  # mixed-zone cols in the high psum half
                for rt in range(NT128):
                    pS_lo = ps.tile([128, 1024], f32, tag="ph")
                    pS_hi = ps.tile([128, 1024], f32, tag="ph")
                    lhsT = ntc[rt // 4][:, (rt % 4) * 128 : (rt % 4 + 1) * 128]
                    for j in range(NT512):
                        half = pS_lo if j < 2 else pS_hi
                        nc.tensor.matmul(
                            half[:, (j % 2) * 512 : (j % 2) * 512 + 512],
                            lhsT,
                            ntc[j][:],
                        )
                    fake_rows = rt < t_lo
                    real_rows = rt >= t_hi
                    # certain-zone reductions (CF <= 1024 <= CR by construction)
                    if not real_rows:  # fake or straddle rows: v0, v1
                        nc.vector.tensor_reduce(
                            stc[:, 0, rt : rt + 1], pS_lo[:, 0:CF], axis=X, op=Alu.min
                        )
                        nc.vector.tensor_reduce(
                            stc[:, 1, rt : rt + 1],
                            pS_hi[:, CR - 1024 : 1024],
                            axis=X,
                            op=Alu.max,
                        )
                        if WL > 0:
                            nc.scalar.copy(stkA[:, rt, 0:WL], pS_lo[:, CF:1024])
                        if WH > 0:
                            nc.scalar.copy(stkA[:, rt, WL:MW], pS_hi[:, 0:WH])
                    if not fake_rows:  # real or straddle rows: v2, v3
                        nc.vector.tensor_reduce(
                            stc[:, 2, rt : rt + 1],
                            pS_hi[:, CR - 1024 : 1024],
                            axis=X,
                            op=Alu.min,
                        )
                        nc.vector.tensor_reduce(
                            stc[:, 3, rt : rt + 1], pS_lo[:, 0:CF], axis=X, op=Alu.max
                        )
                        if WL > 0:
                            nc.scalar.copy(stkB[:, rt - t_lo, 0:WL], pS_lo[:, CF:1024])
                        if WH > 0:
                            nc.scalar.copy(stkB[:, rt - t_lo, WL:MW], pS_hi[:, 0:WH])

                nc.vector.tensor_tensor(
                    stkA[:], stkA[:], bcast_mid(mk[:, 0, :], NA), op=Alu.add
                )
                nc.vector.tensor_reduce(
                    stm[:, 0, 0:NA], stkA[:], axis=X, op=Alu.min
                )
                nc.vector.tensor_reduce(
                    stm[:, 1, 0:NA], stkA[:], axis=X, op=Alu.max
                )
                nc.vector.tensor_tensor(
                    stkB[:], stkB[:], bcast_mid(mk[:, 1, :], NB), op=Alu.add
                )
                nc.vector.tensor_reduce(
                    stm[:, 2, t_lo:NT128], stkB[:], axis=X, op=Alu.min
                )
                nc.vector.tensor_reduce(
                    stm[:, 3, t_lo:NT128], stkB[:], axis=X, op=Alu.max
                )

                nc.gpsimd.dma_start(stats_c[s], stc[:])
                nc.gpsimd.dma_start(stats_m[s], stm[:])

    nc.compile()
    return nc


def kernel(embeddings, label):
    embeddings = np.ascontiguousarray(np.asarray(embeddings, dtype=np.float32))
    label = np.asarray(label)
    assert embeddings.shape == (B, T, D) and label.shape == (B, T)

    # host-side packing: fake (label 0) rows first, per batch
    perms = np.empty((B, T), dtype=np.int64)
    nfs = np.empty(B, dtype=np.int64)
    for b in range(B):
        lb = label[b]
        perms[b] = np.argsort(lb, kind="stable")
        nfs[b] = int((lb == 0).sum())
    valid = (nfs > 0) & (nfs < T)
    if not valid.any():
        return np.float32(0.0)

    CF = int(nfs[valid].min())
    CR = int(nfs[valid].max())
    # the kernel reduces certain zones inside [128, 1024] psum halves, so the
    # mixed zone must bracket column 1024; invalid batches run through the
    # device with garbage-safe ranges
    CF = max(8, min(CF, 1024))
    CR = min(T - 8, max(CR, 1024))
    MW = CR - CF
    t_lo = CF // 128
    t_hi = (CR + 127) // 128

    nc = _build(CF, CR, t_lo, t_hi)

    # per-core inputs
    in_maps = []
    packedE = np.empty((B, 128, T), dtype=np.float32)
    for b in range(B):
        packedE[b] = embeddings[b][perms[b]].T  # [D, T]
    for c in range(NCORES):
        embt = np.empty((BPC, 128, T), dtype=np.float32)
        mks = np.zeros((BPC, 128, 2, MW), dtype=np.float32)
        for s in range(BPC):
            b = c * BPC + s
            embt[s] = packedE[b]
            lb_packed = label[b][perms[b]]  # 0..0 1..1
            mz = lb_packed[CF:CR]
            mks[s, :, 0, :] = np.where(mz == 1, BIG, 0.0)[None, :]  # biased_A: +16 on real
            mks[s, :, 1, :] = np.where(mz == 0, BIG, 0.0)[None, :]  # biased_B: +16 on fake
        in_maps.append({"embt": embt, "masks": mks})

    from concourse.bass_utils import run_bass_kernel_spmd

    trace = bool(os.environ.get("CRL_TRACE"))
    if trace:
        _install_ntff_shim()
    res = run_bass_kernel_spmd(
        nc, in_maps, core_ids=list(range(NCORES)), trace=trace
    )
    if trace and res.exec_time_ns is not None:
        print(f"HW exec time: {res.exec_time_ns} ns")
        if res.instructions_and_trace:
            print("trace:", res.instructions_and_trace[1])

    # host tail on [T]-sized stats
    total = 0.0
    for c in range(NCORES):
        out = res.results[c]
        for s in range(BPC):
            b = c * BPC + s
            if not valid[b]:
                continue
            nf = int(nfs[b])
            stc = out["stats_c"][s].astype(np.float64)  # [128, 4, NT]
            stm = out["stats_m"][s].astype(np.float64)
            # row r = t*128 + p  ->  [4, T]
            sc = stc.transpose(1, 2, 0).reshape(4, T)
            sm = stm.transpose(1, 2, 0).reshape(4, T)
            minfake = np.minimum(sc[0], sm[0])
            maxreal = np.maximum(sc[1], sm[1] - BIG)
            minreal = np.minimum(sc[2], sm[2])
            maxfake = np.maximum(sc[3], sm[3] - BIG)
            f2f = np.maximum(TH_SIM_MIN - minfake[:nf], 0.0).mean()
            r2r = np.maximum(TH_SIM_MIN - minreal[nf:], 0.0).mean()
            f2r = np.maximum(maxreal[:nf] - TH_DIFF_MAX, 0.0).mean()
            r2f = np.maximum(maxfake[nf:] - TH_DIFF_MAX, 0.0).mean()
            total += f2f + r2r + f2r + r2f
    return np.float32(total / B)


def _install_ntff_shim():
    """antenv.axon_hooks is missing on this image; inject it so trace=True works."""
    import types

    import antenv

    if hasattr(antenv, "axon_hooks"):
        return
    from trn_agent_boot.trn_boot import _ntff_profile_via_ctypes

    mod = types.ModuleType("antenv.axon_hooks")
    mod._hook = _ntff_profile_via_ctypes("/opt/axon/libaxon_pjrt.so")
    mod.get_axon_ntff_profile_hook = lambda: mod._hook
    mod.set_axon_ntff_profile_hook = lambda h: setattr(mod, "_hook", h)
    sys.modules["antenv.axon_hooks"] = mod
    antenv.axon_hooks = mod
